# revision 47
# baseline (speedup 1.0000x reference)
"""Bidirectional GQA attention block (B=2, S=4096, D=768, 6 Q heads / 2 KV heads,
head_dim=128) on 8 Trainium2 NeuronCores.

Sharding: core = b*4 + kvh*2 + sh
  b   in {0,1}: batch            (data parallel)
  kvh in {0,1}: kv-head group    (tensor parallel: 3 q-heads + 1 kv head each)
  sh  in {0,1}: query half       (sequence parallel on queries)
Each core computes K/V for its kv head over the full sequence, Q for its
2048-query chunk and 3 heads, unnormalized attention output transposed
(e x q), folds softmax normalization into a post-scale, and projects through
its 384 rows of wo.  Host sums the two kv-group partials per (b, sh).

v2: fully-fused single-pass schedule.  The projection matmuls (K, V, Q, wo)
are emitted as *filler* between attention score groups, so the PE stream is
one continuous sequence of back-to-back matmuls and the ACT (exp) engine is
fed from ~5us onward instead of idling through a separate projection phase.
Score groups are 2 k-tiles (PSUM: 2x2-bank sps double buffer + 1-bank AV
accumulator + 3-bank filler pool = 8 banks).  exp outputs land in a
persistent per-block pT buffer [128, 32, 512]; the softmax denominator is
computed with wide tree adds over that buffer (4 subtree sums of 8 tiles,
then 3 combines) instead of 32 chained adds, cutting DVE busy time.  The
final block uses the incremental chain so its tail stays short.  A handful
of warm-up matmuls + a dummy exp run during the initial DMAs to warm the
HAM clock gate and preload the ACT exp table.
"""

import numpy as np
import ml_dtypes

import concourse.bass as bass
import concourse.mybir as mybir
import concourse.tile as tile
from concourse import bacc
from concourse.bass_utils import run_bass_kernel_spmd

# problem constants (hardcoded; harness supplies exactly these shapes)
B, S, D = 2, 4096, 768
N_HEADS, N_KV, HD = 6, 2, 128
GH = N_HEADS // N_KV          # q-heads per kv group = 3
QC = S // 2                   # per-core query chunk = 2048
P = 128                       # partitions
NB = D // P                   # 6 contraction blocks
ST = S // P                   # 32 key tiles
SC = 512                      # s-chunk for projections
QB = 512                      # q block in attention
NG = 16                       # score groups per block (2 k-tiles each)
GSZ = 2
SCALE = 1.0 / float(np.sqrt(HD))

FP32 = mybir.dt.float32
BF16 = mybir.dt.bfloat16
BF = ml_dtypes.bfloat16


def _emit(tc, xT, wq3, wk1, wv1, wo3, y):
    nc = tc.nc
    Exp = mybir.ActivationFunctionType.Exp

    with tc.tile_pool(name="persist", bufs=1) as persist, \
         tc.tile_pool(name="px", bufs=3) as px, \
         tc.tile_pool(name="pxq", bufs=4) as pxq, \
         tc.tile_pool(name="psps", bufs=2, space="PSUM") as psps, \
         tc.tile_pool(name="pav", bufs=2, space="PSUM") as pav, \
         tc.tile_pool(name="pfil", bufs=2, space="PSUM") as pfil, \
         tc.tile_pool(name="psb", bufs=3) as psb:

        # ---------------- persistent tiles ----------------
        kT = persist.tile([P, S], BF16)            # K^T [e, ks]
        vS = persist.tile([P, ST, HD], BF16)       # V   [s%128, ks-tile, e]
        qT = persist.tile([P, GH, QC], BF16)       # Q^T [e, h, q]
        attT = persist.tile([P, GH, QC], BF16)     # normalized attn^T
        wo_s = persist.tile([P, GH, D], BF16)
        wq_s = persist.tile([P, NB, GH * HD], BF16)
        wk_s = persist.tile([P, NB, HD], BF16)
        wv_s = persist.tile([P, NB, HD], BF16)
        ones_sq = persist.tile([P, P], BF16)
        junk = persist.tile([P, QB], BF16)
        junk2 = persist.tile([P, 8], BF16)
        pT0 = persist.tile([P, ST, QB], BF16)      # exp'd scores, even blocks
        pT1 = persist.tile([P, ST, QB], BF16)      # exp'd scores, odd blocks
        pT_bufs = [pT0, pT1]

        # ---------------- initial DMAs (priority order) ----------------
        nc.sync.dma_start(out=wk_s, in_=wk1)
        xt_tiles = {}
        xtq_tiles = {}

        def issue_xt(c, split=False):
            # keys are permuted per-core so chunks 0-3 are this core's own
            # query half: they feed BOTH the Q projection and K/V, and live
            # in the long-lived pxq pool (4 allocs, never recycled).
            pool = pxq if c < 4 else px
            t = pool.tile([P, NB, SC], BF16, tag="xq" if c < 4 else "xt",
                          name=f"xt_{c}")
            if split:
                for db in range(0, NB, 2):
                    nc.sync.dma_start(out=t[:, db:db + 2, :],
                                      in_=xT[c][:, db:db + 2, :])
            else:
                nc.sync.dma_start(out=t, in_=xT[c])
            xt_tiles[c] = t

        nc.sync.dma_start(out=wv_s, in_=wv1)
        issue_xt(0, split=True)
        nc.sync.dma_start(out=wq_s, in_=wq3)
        issue_xt(1)
        issue_xt(2)
        issue_xt(3)

        # ---------------- warmup (HAM clock gate + ACT table) ----------
        nc.vector.memset(ones_sq, 1.0)
        nc.vector.memset(junk, 0.0)
        wps_a = pfil.tile([P, QB], FP32, tag="fil", name="warm_a")
        for w in range(4):
            nc.tensor.matmul(wps_a, lhsT=ones_sq, rhs=junk,
                             start=w == 0, stop=w == 3)
        # dummy exp preloads the ACT spline table (~2.7us) early
        nc.scalar.activation(junk2, wps_a[:, :8], Exp, scale=SCALE)
        wps_b = pfil.tile([P, QB], FP32, tag="fil", name="warm_b")
        for w in range(4):
            nc.tensor.matmul(wps_b, lhsT=ones_sq, rhs=junk,
                             start=w == 0, stop=w == 3)
        nc.vector.tensor_copy(junk[:, :8], wps_b[:, :8])

        # ---------------- filler item emitters ----------------
        # emit_*a starts a projection (first 3 contraction blocks into a fil
        # PSUM tile); emit_*b finishes it (last 3 blocks + evacuation copy).
        # Splitting keeps per-slot filler load ~uniform so the ACT engine is
        # never starved by a 6-matmul lump.
        half = {}

        def emit_ka(c):
            kps = pfil.tile([P, SC], FP32, tag="fil", name=f"kps_{c}")
            xt = xt_tiles[c]
            for db in range(3):
                nc.tensor.matmul(kps, lhsT=wk_s[:, db, :], rhs=xt[:, db, :],
                                 start=db == 0, stop=False)
            half[("k", c)] = kps

        def emit_kb(c):
            kps = half.pop(("k", c))
            xt = xt_tiles[c]
            for db in range(3, NB):
                nc.tensor.matmul(kps, lhsT=wk_s[:, db, :], rhs=xt[:, db, :],
                                 start=False, stop=db == NB - 1)
            nc.vector.tensor_copy(kT[:, c * SC:(c + 1) * SC], kps)

        def emit_k(c):
            emit_ka(c)
            emit_kb(c)

        def emit_v(t):
            c = t // 4
            t4 = t % 4
            vps = pfil.tile([P, HD], FP32, tag="fil", name=f"vps_{t}")
            xt = xt_tiles[c]
            for db in range(NB):
                nc.tensor.matmul(vps, lhsT=xt[:, db, t4 * P:(t4 + 1) * P],
                                 rhs=wv_s[:, db, :],
                                 start=db == 0, stop=db == NB - 1)
            nc.vector.tensor_copy(vS[:, t, :], vps)

        def emit_qa(h, qc):
            qps = pfil.tile([P, SC], FP32, tag="fil", name=f"qps_{h}_{qc}")
            xtq = xt_tiles[qc]
            for db in range(3):
                nc.tensor.matmul(qps, lhsT=wq_s[:, db, h * HD:(h + 1) * HD],
                                 rhs=xtq[:, db, :],
                                 start=db == 0, stop=False)
            half[("q", h, qc)] = qps

        def emit_qb(h, qc):
            qps = half.pop(("q", h, qc))
            xtq = xt_tiles[qc]
            for db in range(3, NB):
                nc.tensor.matmul(qps, lhsT=wq_s[:, db, h * HD:(h + 1) * HD],
                                 rhs=xtq[:, db, :],
                                 start=False, stop=db == NB - 1)
            nc.vector.tensor_copy(qT[:, h, qc * SC:(qc + 1) * SC], qps)

        def emit_q(h, qc):
            emit_qa(h, qc)
            emit_qb(h, qc)

        def emit_wo_half(qb, qt4, c0, cn):
            qt = qb * (QB // P) + qt4
            if c0 == 0:
                ysb = psb.tile([P, D], FP32, tag="ysb", bufs=3,
                               name=f"ysb_{qt}")
                half[("y", qt)] = ysb
            else:
                ysb = half.pop(("y", qt))
            yps = pfil.tile([P, 512], FP32, tag="fil", name=f"yps_{qt}_{c0}")
            for eb in range(GH):
                nc.tensor.matmul(yps[:, :cn],
                                 lhsT=attT[:, eb, qt * P:(qt + 1) * P],
                                 rhs=wo_s[:, eb, c0:c0 + cn],
                                 start=eb == 0, stop=eb == GH - 1)
            nc.vector.tensor_copy(ysb[:, c0:c0 + cn], yps[:, :cn])
            if c0 != 0:
                nc.sync.dma_start(out=y[qt * P:(qt + 1) * P, :], in_=ysb)

        def emit_wo(qb, qt4):
            emit_wo_half(qb, qt4, 0, 512)
            emit_wo_half(qb, qt4, 512, 256)

        # ---------------- static filler schedule ----------------
        # filler[(bi, g)] = list of closures to emit after scores(bi, g)
        filler = {}

        def add_f(bi, g, fn):
            filler.setdefault((bi, g), []).append(fn)

        # block 0: K chunks c1..c7 + V tile pairs (AV(g) needs tiles 2g,2g+1;
        # pair for AV(g+2) is emitted at slot g) + Q heads 1,2 of chunk 0.
        for j in range(1, 8):
            def k_item(j=j):
                emit_k(j)
                if 4 <= j + 1 <= 7:
                    issue_xt(j + 1)
            add_f(0, 2 * j - 2, k_item)
        for g in range(16):
            def v_item(g=g):
                emit_v(2 * g)
                emit_v(2 * g + 1)
            add_f(0, g, v_item)
        add_f(0, 13, lambda: emit_q(1, 0))
        add_f(0, 14, lambda: emit_q(2, 0))
        add_f(1, 0, lambda: nc.sync.dma_start(out=wo_s, in_=wo3))

        def add_q(bi, g, h, qc):
            add_f(bi, g, lambda: emit_qa(h, qc))
            add_f(bi, g + 1, lambda: emit_qb(h, qc))

        def add_wo(bi, g, qb, qt4):
            add_f(bi, g, lambda: emit_wo_half(qb, qt4, 0, 512))
            add_f(bi, g + 1, lambda: emit_wo_half(qb, qt4, 512, 256))

        # Q projections for later chunks (deadline: qT[h, qc] before block
        # 3*qc + h), plus xTq prefetch
        add_q(1, 2, 0, 1)
        add_q(1, 8, 1, 1)
        add_q(2, 2, 2, 1)
        add_q(2, 12, 0, 2)
        add_q(4, 2, 1, 2)
        add_q(5, 2, 2, 2)
        add_q(7, 12, 0, 3)
        add_q(8, 2, 1, 3)
        add_q(8, 12, 2, 3)
        # wo fillers: wo(qb) ready after tail of block 3*qb+2, which is
        # emitted at slot (3*qb+3, 1); give the DVE tail chain a few slots.
        add_wo(3, 4, 0, 0)
        add_wo(3, 10, 0, 1)
        add_wo(4, 8, 0, 2)
        add_wo(4, 13, 0, 3)
        add_wo(6, 4, 1, 0)
        add_wo(6, 10, 1, 1)
        add_wo(7, 4, 1, 2)
        add_wo(7, 10, 1, 3)
        add_wo(9, 4, 2, 0)
        add_wo(9, 10, 2, 1)
        add_wo(10, 4, 2, 2)
        add_wo(10, 10, 2, 3)

        # ---------------- preamble compute ----------------
        # V tiles before Q: the Q chunk's xTq DMA is last in the
        # bandwidth-bound initial transfer burst, so Q goes last.
        emit_k(0)
        emit_q(0, 0)

        # ---------------- fused attention blocks ----------------
        blocks = [(qb, h) for qb in range(QC // QB) for h in range(GH)]
        state = {}

        def emit_scores(bi, g):
            qb, h = blocks[bi]
            qsl = slice(qb * QB, (qb + 1) * QB)
            pTb = pT_bufs[bi % 2]
            sps = psps.tile([P, GSZ, QB], FP32, tag="sps", name=f"sps_{bi}_{g}")
            for t in range(GSZ):
                kst = GSZ * g + t
                nc.tensor.matmul(sps[:, t, :],
                                 lhsT=kT[:, kst * P:(kst + 1) * P],
                                 rhs=qT[:, h, qsl],
                                 start=True, stop=True)
            nc.scalar.activation(pTb[:, GSZ * g:GSZ * (g + 1), :], sps,
                                 Exp, scale=SCALE)

        def emit_av(bi, g):
            qb, h = blocks[bi]
            st = state.setdefault(bi, {"avps": None, "dacc": [None, None]})
            pTb = pT_bufs[bi % 2]
            if st["avps"] is None:
                st["avps"] = pav.tile([P, QB], FP32, tag="av",
                                      name=f"avps_{bi}")
            for t in range(GSZ):
                kst = GSZ * g + t
                nc.tensor.matmul(st["avps"], lhsT=vS[:, kst, :],
                                 rhs=pTb[:, kst, :],
                                 start=kst == 0, stop=kst == ST - 1)
                if bi == len(blocks) - 1:
                    # final block: incremental denominator chain (short tail)
                    par = kst & 1
                    dnew = psb.tile([P, QB], BF16, tag=f"dacc{par}", bufs=2,
                                    name=f"dacc_{bi}_{kst}")
                    if kst < 2:
                        nc.vector.tensor_copy(dnew, pTb[:, kst, :])
                    else:
                        nc.vector.tensor_add(dnew, st["dacc"][par],
                                             pTb[:, kst, :])
                    st["dacc"][par] = dnew

        def emit_subtree(bi, k):
            # sum pT tiles 8k..8k+7 of block bi -> s3 [P, QB]
            pTb = pT_bufs[bi % 2]
            st = state.setdefault(bi, {"avps": None, "dacc": [None, None]})
            s1 = psb.tile([P, 4, QB], BF16, tag="tr4", bufs=2,
                          name=f"tr4_{bi}_{k}")
            nc.vector.tensor_add(s1, pTb[:, 8 * k:8 * k + 4, :],
                                 pTb[:, 8 * k + 4:8 * k + 8, :])
            s2 = psb.tile([P, 2, QB], BF16, tag="tr2", bufs=2,
                          name=f"tr2_{bi}_{k}")
            nc.vector.tensor_add(s2, s1[:, 0:2, :], s1[:, 2:4, :])
            s3 = psb.tile([P, QB], BF16, tag="tr1", bufs=5,
                          name=f"tr1_{bi}_{k}")
            nc.vector.tensor_add(s3, s2[:, 0, :], s2[:, 1, :])
            st.setdefault("sub", []).append(s3)
            if k == 1:
                c1 = psb.tile([P, QB], BF16, tag="trc", bufs=2,
                              name=f"trc_{bi}")
                nc.vector.tensor_add(c1, st["sub"][0], st["sub"][1])
                st["c1"] = c1

        def emit_tail(bi):
            qb, h = blocks[bi]
            qsl = slice(qb * QB, (qb + 1) * QB)
            st = state.pop(bi)
            avcp = psb.tile([P, QB], FP32, tag="avcp", bufs=2,
                            name=f"avcp_{bi}")
            nc.vector.tensor_copy(avcp, st["avps"])
            if bi == len(blocks) - 1:
                dsum = psb.tile([P, QB], BF16, tag="dacc0", bufs=2,
                                name=f"dsum_{bi}")
                nc.vector.tensor_add(dsum, st["dacc"][0], st["dacc"][1])
            else:
                c2 = psb.tile([P, QB], BF16, tag="trc", bufs=2,
                              name=f"c2_{bi}")
                nc.vector.tensor_add(c2, st["sub"][2], st["sub"][3])
                dsum = psb.tile([P, QB], BF16, tag="dsum", bufs=2,
                                name=f"dsum_{bi}")
                nc.vector.tensor_add(dsum, st["c1"], c2)
            den_b = pfil.tile([P, QB], FP32, tag="fil", name=f"den_{bi}")
            nc.tensor.matmul(den_b, lhsT=ones_sq, rhs=dsum,
                             start=True, stop=True)
            rb = psb.tile([P, QB], FP32, tag="rb", bufs=3, name=f"rb_{bi}")
            nc.vector.reciprocal_approx_fast(rb, den_b)
            nc.vector.tensor_mul(attT[:, h, qsl], avcp, rb)

        # software pipeline: scores(g) | filler(g) | AV(g-trail).  AV trails
        # the scores/exp by 2 groups (4 during the filler-heavy block 0) so
        # the AV matmuls never wait on exp completion at rate-tie — the
        # exp(g) -> AV(g) semaphore latency hides behind two group periods.
        from collections import deque
        pend = deque()

        def drain(pbi, pg):
            emit_av(pbi, pg)
            if pg % 4 == 3 and pbi != len(blocks) - 1:
                emit_subtree(pbi, pg // 4)
            if pg == NG - 1:
                emit_tail(pbi)

        for bi in range(len(blocks)):
            depth = 4 if bi == 0 else 2
            for g in range(NG):
                emit_scores(bi, g)
                for fn in filler.get((bi, g), ()):
                    fn()
                pend.append((bi, g))
                while len(pend) > depth:
                    drain(*pend.popleft())
        while pend:
            drain(*pend.popleft())
        for qt4 in range(4):
            emit_wo(3, qt4)


def _build_nc():
    nc = bacc.Bacc("TRN2", target_bir_lowering=False, debug=False, num_devices=8)
    xT = nc.dram_tensor("xT", [S // SC, P, NB, SC], BF16, kind="ExternalInput").ap()
    wq3 = nc.dram_tensor("wq3", [P, NB, GH * HD], BF16, kind="ExternalInput").ap()
    wk1 = nc.dram_tensor("wk1", [P, NB, HD], BF16, kind="ExternalInput").ap()
    wv1 = nc.dram_tensor("wv1", [P, NB, HD], BF16, kind="ExternalInput").ap()
    wo3 = nc.dram_tensor("wo3", [P, GH, D], BF16, kind="ExternalInput").ap()
    y = nc.dram_tensor("y", [QC, D], FP32, kind="ExternalOutput").ap()
    with tile.TileContext(nc) as tc:
        _emit(tc, xT, wq3, wk1, wv1, wo3, y)
    nc.compile()
    return nc


_NC = None


def _get_nc():
    global _NC
    if _NC is None:
        _NC = _build_nc()
    return _NC


def make_in_maps(x, wq, wk, wv, wo):
    x = np.asarray(x, np.float32)
    in_maps = []
    for core in range(8):
        b, kvh, sh = core >> 2, (core >> 1) & 1, core & 1
        xTb_n = x[b].T.astype(BF)                    # [D, S]
        # per-core key permutation: own query-half columns first (softmax
        # over keys is permutation invariant; outputs index queries only)
        xTb = np.concatenate(
            [xTb_n[:, sh * QC:(sh + 1) * QC],
             xTb_n[:, (1 - sh) * QC:(2 - sh) * QC]], axis=1)
        g0, g1 = kvh * GH * HD, (kvh + 1) * GH * HD

        def tile_dm(a):                              # [D, M] -> [P, NB, M]
            return np.ascontiguousarray(
                a.reshape(NB, P, a.shape[1]).transpose(1, 0, 2))

        def tile_x(a):                               # [D, M] -> [M/SC, P, NB, SC]
            return np.ascontiguousarray(
                a.reshape(NB, P, a.shape[1] // SC, SC).transpose(2, 1, 0, 3))

        in_maps.append({
            "xT": tile_x(xTb),
            "wq3": tile_dm(np.asarray(wq, np.float32)[:, g0:g1].astype(BF)),
            "wk1": tile_dm(np.asarray(wk, np.float32)[:, kvh * HD:(kvh + 1) * HD].astype(BF)),
            "wv1": tile_dm(np.asarray(wv, np.float32)[:, kvh * HD:(kvh + 1) * HD].astype(BF)),
            "wo3": np.ascontiguousarray(
                np.asarray(wo, np.float32)[g0:g1, :].astype(BF)
                .reshape(GH, P, D).transpose(1, 0, 2)),
        })
    return in_maps


def combine_outputs(results):
    """results: list of 8 per-core {name: array} dicts -> full [B, S, D] output."""
    y = np.zeros((B, S, D), np.float32)
    for b in range(B):
        for sh in range(2):
            c0 = b * 4 + 0 * 2 + sh
            c1 = b * 4 + 1 * 2 + sh
            y[b, sh * QC:(sh + 1) * QC, :] = (
                results[c0]["y"].astype(np.float32)
                + results[c1]["y"].astype(np.float32)
            )
    return y


def kernel(x, wq, wk, wv, wo, **run_kwargs):
    nc = _get_nc()
    in_maps = make_in_maps(x, wq, wk, wv, wo)
    res = run_bass_kernel_spmd(nc, in_maps, core_ids=list(range(8)), **run_kwargs)
    out = combine_outputs(res.results)
    if run_kwargs:
        kernel.last_result = res
    return out


if __name__ == "__main__":
    rng = np.random.default_rng(0)
    x = rng.standard_normal((B, S, D), dtype=np.float32)
    std = 1.0 / np.sqrt(D)
    wq = rng.standard_normal((D, N_HEADS * HD), dtype=np.float32) * std
    wk = rng.standard_normal((D, N_KV * HD), dtype=np.float32) * std
    wv = rng.standard_normal((D, N_KV * HD), dtype=np.float32) * std
    wo = rng.standard_normal((N_HEADS * HD, D), dtype=np.float32) * std
    y = kernel(x, wq, wk, wv, wo)
    print("kernel output", y.shape, y.dtype, float(np.abs(y).max()))


# revision 48
# speedup vs baseline: 1.0272x; 1.0272x over previous
"""Bidirectional GQA attention block (B=2, S=4096, D=768, 6 Q heads / 2 KV heads,
head_dim=128) on 8 Trainium2 NeuronCores.

Sharding: core = b*4 + kvh*2 + sh
  b   in {0,1}: batch            (data parallel)
  kvh in {0,1}: kv-head group    (tensor parallel: 3 q-heads + 1 kv head each)
  sh  in {0,1}: query half       (sequence parallel on queries)
Each core computes K/V for its kv head over the full sequence, Q for its
2048-query chunk and 3 heads, unnormalized attention output transposed
(e x q), folds softmax normalization into a post-scale, and projects through
its 384 rows of wo.  Host sums the two kv-group partials per (b, sh).

v2: fully-fused single-pass schedule.  The projection matmuls (K, V, Q, wo)
are emitted as *filler* between attention score groups, so the PE stream is
one continuous sequence of back-to-back matmuls and the ACT (exp) engine is
fed from ~5us onward instead of idling through a separate projection phase.
Score groups are 2 k-tiles (PSUM: 2x2-bank sps double buffer + 1-bank AV
accumulator + 3-bank filler pool = 8 banks).  exp outputs land in a
persistent per-block pT buffer [128, 32, 512]; the softmax denominator is
computed with wide tree adds over that buffer (4 subtree sums of 8 tiles,
then 3 combines) instead of 32 chained adds, cutting DVE busy time.  The
final block uses the incremental chain so its tail stays short.  A handful
of warm-up matmuls + a dummy exp run during the initial DMAs to warm the
HAM clock gate and preload the ACT exp table.
"""

import numpy as np
import ml_dtypes

import concourse.bass as bass
import concourse.mybir as mybir
import concourse.tile as tile
from concourse import bacc
from concourse.bass_utils import run_bass_kernel_spmd

# problem constants (hardcoded; harness supplies exactly these shapes)
B, S, D = 2, 4096, 768
N_HEADS, N_KV, HD = 6, 2, 128
GH = N_HEADS // N_KV          # q-heads per kv group = 3
QC = S // 2                   # per-core query chunk = 2048
P = 128                       # partitions
NB = D // P                   # 6 contraction blocks
ST = S // P                   # 32 key tiles
SC = 512                      # s-chunk for projections
QB = 512                      # q block in attention
NG = 16                       # score groups per block (2 k-tiles each)
GSZ = 2
SCALE = 1.0 / float(np.sqrt(HD))

FP32 = mybir.dt.float32
BF16 = mybir.dt.bfloat16
BF = ml_dtypes.bfloat16


def _emit(tc, xT, wq3, wk1, wv1, wo3, y):
    nc = tc.nc
    Exp = mybir.ActivationFunctionType.Exp

    with tc.tile_pool(name="persist", bufs=1) as persist, \
         tc.tile_pool(name="px", bufs=3) as px, \
         tc.tile_pool(name="pxq", bufs=4) as pxq, \
         tc.tile_pool(name="psps", bufs=2, space="PSUM") as psps, \
         tc.tile_pool(name="pav", bufs=2, space="PSUM") as pav, \
         tc.tile_pool(name="pfil", bufs=2, space="PSUM") as pfil, \
         tc.tile_pool(name="psb", bufs=3) as psb:

        # ---------------- persistent tiles ----------------
        kT = persist.tile([P, S], BF16)            # K^T [e, ks]
        vS = persist.tile([P, ST, HD], BF16)       # V   [s%128, ks-tile, e]
        qT = persist.tile([P, GH, QC], BF16)       # Q^T [e, h, q]
        attT = persist.tile([P, GH, QC], BF16)     # normalized attn^T
        wo_s = persist.tile([P, GH, D], BF16)
        wq_s = persist.tile([P, NB, GH * HD], BF16)
        wk_s = persist.tile([P, NB, HD], BF16)
        wv_s = persist.tile([P, NB, HD], BF16)
        ones_sq = persist.tile([P, P], BF16)
        junk = persist.tile([P, QB], BF16)
        junk2 = persist.tile([P, 8], BF16)
        pT0 = persist.tile([P, ST, QB], BF16)      # exp'd scores, even blocks
        pT1 = persist.tile([P, ST, QB], BF16)      # exp'd scores, odd blocks
        pT_bufs = [pT0, pT1]

        # ---------------- initial DMAs (priority order) ----------------
        nc.sync.dma_start(out=wk_s, in_=wk1)
        xt_tiles = {}
        xtq_tiles = {}

        def issue_xt(c, split=False):
            # keys are permuted per-core so chunks 0-3 are this core's own
            # query half: they feed BOTH the Q projection and K/V, and live
            # in the long-lived pxq pool (4 allocs, never recycled).
            pool = pxq if c < 4 else px
            t = pool.tile([P, NB, SC], BF16, tag="xq" if c < 4 else "xt",
                          name=f"xt_{c}")
            if split:
                for db in range(0, NB, 2):
                    nc.sync.dma_start(out=t[:, db:db + 2, :],
                                      in_=xT[c][:, db:db + 2, :])
            else:
                nc.sync.dma_start(out=t, in_=xT[c])
            xt_tiles[c] = t

        nc.sync.dma_start(out=wv_s, in_=wv1)
        issue_xt(0, split=True)
        nc.sync.dma_start(out=wq_s, in_=wq3)
        issue_xt(1)
        issue_xt(2)
        issue_xt(3)

        # ---------------- warmup (HAM clock gate + ACT table) ----------
        nc.vector.memset(ones_sq, 1.0)
        nc.vector.memset(junk, 0.0)
        wps_a = pfil.tile([P, QB], FP32, tag="fil", name="warm_a")
        for w in range(4):
            nc.tensor.matmul(wps_a, lhsT=ones_sq, rhs=junk,
                             start=w == 0, stop=w == 3)
        # dummy exp preloads the ACT spline table (~2.7us) early
        nc.scalar.activation(junk2, wps_a[:, :8], Exp, scale=SCALE)
        wps_b = pfil.tile([P, QB], FP32, tag="fil", name="warm_b")
        for w in range(4):
            nc.tensor.matmul(wps_b, lhsT=ones_sq, rhs=junk,
                             start=w == 0, stop=w == 3)
        nc.vector.tensor_copy(junk[:, :8], wps_b[:, :8])

        # ---------------- filler item emitters ----------------
        # emit_*a starts a projection (first 3 contraction blocks into a fil
        # PSUM tile); emit_*b finishes it (last 3 blocks + evacuation copy).
        # Splitting keeps per-slot filler load ~uniform so the ACT engine is
        # never starved by a 6-matmul lump.
        half = {}

        def emit_ka(c):
            kps = pfil.tile([P, SC], FP32, tag="fil", name=f"kps_{c}")
            xt = xt_tiles[c]
            for db in range(3):
                nc.tensor.matmul(kps, lhsT=wk_s[:, db, :], rhs=xt[:, db, :],
                                 start=db == 0, stop=False)
            half[("k", c)] = kps

        def emit_kb(c):
            kps = half.pop(("k", c))
            xt = xt_tiles[c]
            for db in range(3, NB):
                nc.tensor.matmul(kps, lhsT=wk_s[:, db, :], rhs=xt[:, db, :],
                                 start=False, stop=db == NB - 1)
            nc.vector.tensor_copy(kT[:, c * SC:(c + 1) * SC], kps)

        def emit_k(c):
            emit_ka(c)
            emit_kb(c)

        def emit_v(t):
            c = t // 4
            t4 = t % 4
            vps = pfil.tile([P, HD], FP32, tag="fil", name=f"vps_{t}")
            xt = xt_tiles[c]
            for db in range(NB):
                nc.tensor.matmul(vps, lhsT=xt[:, db, t4 * P:(t4 + 1) * P],
                                 rhs=wv_s[:, db, :],
                                 start=db == 0, stop=db == NB - 1)
            nc.vector.tensor_copy(vS[:, t, :], vps)

        def emit_qa(h, qc):
            qps = pfil.tile([P, SC], FP32, tag="fil", name=f"qps_{h}_{qc}")
            xtq = xt_tiles[qc]
            for db in range(3):
                nc.tensor.matmul(qps, lhsT=wq_s[:, db, h * HD:(h + 1) * HD],
                                 rhs=xtq[:, db, :],
                                 start=db == 0, stop=False)
            half[("q", h, qc)] = qps

        def emit_qb(h, qc):
            qps = half.pop(("q", h, qc))
            xtq = xt_tiles[qc]
            for db in range(3, NB):
                nc.tensor.matmul(qps, lhsT=wq_s[:, db, h * HD:(h + 1) * HD],
                                 rhs=xtq[:, db, :],
                                 start=False, stop=db == NB - 1)
            nc.vector.tensor_copy(qT[:, h, qc * SC:(qc + 1) * SC], qps)

        def emit_q(h, qc):
            emit_qa(h, qc)
            emit_qb(h, qc)

        def emit_wo_half(qb, qt4, c0, cn):
            qt = qb * (QB // P) + qt4
            if c0 == 0:
                ysb = psb.tile([P, D], FP32, tag="ysb", bufs=3,
                               name=f"ysb_{qt}")
                half[("y", qt)] = ysb
            else:
                ysb = half.pop(("y", qt))
            yps = pfil.tile([P, 512], FP32, tag="fil", name=f"yps_{qt}_{c0}")
            for eb in range(GH):
                nc.tensor.matmul(yps[:, :cn],
                                 lhsT=attT[:, eb, qt * P:(qt + 1) * P],
                                 rhs=wo_s[:, eb, c0:c0 + cn],
                                 start=eb == 0, stop=eb == GH - 1)
            nc.vector.tensor_copy(ysb[:, c0:c0 + cn], yps[:, :cn])
            if c0 != 0:
                nc.sync.dma_start(out=y[qt * P:(qt + 1) * P, :], in_=ysb)

        def emit_wo(qb, qt4):
            emit_wo_half(qb, qt4, 0, 512)
            emit_wo_half(qb, qt4, 512, 256)

        # ---------------- static filler schedule ----------------
        # filler[(bi, g)] = list of closures to emit after scores(bi, g)
        filler = {}

        def add_f(bi, g, fn):
            filler.setdefault((bi, g), []).append(fn)

        # block 0: K chunks c1..c7 + V tile pairs (AV(g) needs tiles 2g,2g+1;
        # pair for AV(g+2) is emitted at slot g) + Q heads 1,2 of chunk 0.
        for j in range(1, 8):
            def k_item(j=j):
                emit_k(j)
                if 4 <= j + 1 <= 7:
                    issue_xt(j + 1)
            add_f(0, 2 * j - 2, k_item)
        for g in range(14):
            def v_item(g=g):
                emit_v(2 * g + 4)
                emit_v(2 * g + 5)
            add_f(0, g, v_item)
        add_f(0, 13, lambda: emit_q(1, 0))
        add_f(0, 14, lambda: emit_q(2, 0))
        add_f(1, 0, lambda: nc.sync.dma_start(out=wo_s, in_=wo3))

        def add_q(bi, g, h, qc):
            add_f(bi, g, lambda: emit_qa(h, qc))
            add_f(bi, g + 1, lambda: emit_qb(h, qc))

        def add_wo(bi, g, qb, qt4):
            add_f(bi, g, lambda: emit_wo_half(qb, qt4, 0, 512))
            add_f(bi, g + 1, lambda: emit_wo_half(qb, qt4, 512, 256))

        # Q projections for later chunks (deadline: qT[h, qc] before block
        # 3*qc + h), plus xTq prefetch
        add_q(1, 2, 0, 1)
        add_q(1, 8, 1, 1)
        add_q(2, 2, 2, 1)
        add_q(2, 12, 0, 2)
        add_q(4, 2, 1, 2)
        add_q(5, 2, 2, 2)
        add_q(7, 12, 0, 3)
        add_q(8, 2, 1, 3)
        add_q(8, 12, 2, 3)
        # wo fillers: wo(qb) ready after tail of block 3*qb+2, which is
        # emitted at slot (3*qb+3, 1); give the DVE tail chain a few slots.
        add_wo(3, 4, 0, 0)
        add_wo(3, 10, 0, 1)
        add_wo(4, 8, 0, 2)
        add_wo(4, 13, 0, 3)
        add_wo(6, 4, 1, 0)
        add_wo(6, 10, 1, 1)
        add_wo(7, 4, 1, 2)
        add_wo(7, 10, 1, 3)
        add_wo(9, 4, 2, 0)
        add_wo(9, 10, 2, 1)
        add_wo(10, 4, 2, 2)
        add_wo(10, 10, 2, 3)

        # ---------------- preamble compute ----------------
        # V tiles before Q: the Q chunk's xTq DMA is last in the
        # bandwidth-bound initial transfer burst, so Q goes last.
        emit_k(0)
        emit_v(0)
        emit_v(1)
        emit_v(2)
        emit_v(3)
        emit_q(0, 0)

        # ---------------- fused attention blocks ----------------
        blocks = [(qb, h) for qb in range(QC // QB) for h in range(GH)]
        state = {}

        def emit_scores(bi, g):
            qb, h = blocks[bi]
            qsl = slice(qb * QB, (qb + 1) * QB)
            pTb = pT_bufs[bi % 2]
            sps = psps.tile([P, GSZ, QB], FP32, tag="sps", name=f"sps_{bi}_{g}")
            for t in range(GSZ):
                kst = GSZ * g + t
                nc.tensor.matmul(sps[:, t, :],
                                 lhsT=kT[:, kst * P:(kst + 1) * P],
                                 rhs=qT[:, h, qsl],
                                 start=True, stop=True)
            nc.scalar.activation(pTb[:, GSZ * g:GSZ * (g + 1), :], sps,
                                 Exp, scale=SCALE)

        def emit_av(bi, g):
            qb, h = blocks[bi]
            st = state.setdefault(bi, {"avps": None, "dacc": [None, None]})
            pTb = pT_bufs[bi % 2]
            if st["avps"] is None:
                st["avps"] = pav.tile([P, QB], FP32, tag="av",
                                      name=f"avps_{bi}")
            for t in range(GSZ):
                kst = GSZ * g + t
                nc.tensor.matmul(st["avps"], lhsT=vS[:, kst, :],
                                 rhs=pTb[:, kst, :],
                                 start=kst == 0, stop=kst == ST - 1)
                if bi == len(blocks) - 1:
                    # final block: incremental denominator chain (short tail)
                    par = kst & 1
                    dnew = psb.tile([P, QB], BF16, tag=f"dacc{par}", bufs=2,
                                    name=f"dacc_{bi}_{kst}")
                    if kst < 2:
                        nc.vector.tensor_copy(dnew, pTb[:, kst, :])
                    else:
                        nc.vector.tensor_add(dnew, st["dacc"][par],
                                             pTb[:, kst, :])
                    st["dacc"][par] = dnew

        def emit_subtree(bi, k):
            # sum pT tiles 8k..8k+7 of block bi -> s3 [P, QB]
            pTb = pT_bufs[bi % 2]
            st = state.setdefault(bi, {"avps": None, "dacc": [None, None]})
            s1 = psb.tile([P, 4, QB], BF16, tag="tr4", bufs=2,
                          name=f"tr4_{bi}_{k}")
            nc.vector.tensor_add(s1, pTb[:, 8 * k:8 * k + 4, :],
                                 pTb[:, 8 * k + 4:8 * k + 8, :])
            s2 = psb.tile([P, 2, QB], BF16, tag="tr2", bufs=2,
                          name=f"tr2_{bi}_{k}")
            nc.vector.tensor_add(s2, s1[:, 0:2, :], s1[:, 2:4, :])
            s3 = psb.tile([P, QB], BF16, tag="tr1", bufs=5,
                          name=f"tr1_{bi}_{k}")
            nc.vector.tensor_add(s3, s2[:, 0, :], s2[:, 1, :])
            st.setdefault("sub", []).append(s3)
            if k == 1:
                c1 = psb.tile([P, QB], BF16, tag="trc", bufs=2,
                              name=f"trc_{bi}")
                nc.vector.tensor_add(c1, st["sub"][0], st["sub"][1])
                st["c1"] = c1

        def emit_tail(bi):
            qb, h = blocks[bi]
            qsl = slice(qb * QB, (qb + 1) * QB)
            st = state.pop(bi)
            avcp = psb.tile([P, QB], FP32, tag="avcp", bufs=2,
                            name=f"avcp_{bi}")
            nc.vector.tensor_copy(avcp, st["avps"])
            if bi == len(blocks) - 1:
                dsum = psb.tile([P, QB], BF16, tag="dacc0", bufs=2,
                                name=f"dsum_{bi}")
                nc.vector.tensor_add(dsum, st["dacc"][0], st["dacc"][1])
            else:
                c2 = psb.tile([P, QB], BF16, tag="trc", bufs=2,
                              name=f"c2_{bi}")
                nc.vector.tensor_add(c2, st["sub"][2], st["sub"][3])
                dsum = psb.tile([P, QB], BF16, tag="dsum", bufs=2,
                                name=f"dsum_{bi}")
                nc.vector.tensor_add(dsum, st["c1"], c2)
            den_b = pfil.tile([P, QB], FP32, tag="fil", name=f"den_{bi}")
            nc.tensor.matmul(den_b, lhsT=ones_sq, rhs=dsum,
                             start=True, stop=True)
            rb = psb.tile([P, QB], FP32, tag="rb", bufs=3, name=f"rb_{bi}")
            nc.vector.reciprocal_approx_fast(rb, den_b)
            nc.vector.tensor_mul(attT[:, h, qsl], avcp, rb)

        # software pipeline: scores(g) | filler(g) | AV(g-trail).  AV trails
        # the scores/exp by 2 groups (4 during the filler-heavy block 0) so
        # the AV matmuls never wait on exp completion at rate-tie — the
        # exp(g) -> AV(g) semaphore latency hides behind two group periods.
        from collections import deque
        pend = deque()

        def drain(pbi, pg):
            emit_av(pbi, pg)
            if pg % 4 == 3 and pbi != len(blocks) - 1:
                emit_subtree(pbi, pg // 4)
            if pg == NG - 1:
                emit_tail(pbi)

        for bi in range(len(blocks)):
            depth = 4 if bi == 0 else 2
            for g in range(NG):
                emit_scores(bi, g)
                for fn in filler.get((bi, g), ()):
                    fn()
                pend.append((bi, g))
                while len(pend) > depth:
                    drain(*pend.popleft())
        while pend:
            drain(*pend.popleft())
        for qt4 in range(4):
            emit_wo(3, qt4)


def _build_nc():
    nc = bacc.Bacc("TRN2", target_bir_lowering=False, debug=False, num_devices=8)
    xT = nc.dram_tensor("xT", [S // SC, P, NB, SC], BF16, kind="ExternalInput").ap()
    wq3 = nc.dram_tensor("wq3", [P, NB, GH * HD], BF16, kind="ExternalInput").ap()
    wk1 = nc.dram_tensor("wk1", [P, NB, HD], BF16, kind="ExternalInput").ap()
    wv1 = nc.dram_tensor("wv1", [P, NB, HD], BF16, kind="ExternalInput").ap()
    wo3 = nc.dram_tensor("wo3", [P, GH, D], BF16, kind="ExternalInput").ap()
    y = nc.dram_tensor("y", [QC, D], FP32, kind="ExternalOutput").ap()
    with tile.TileContext(nc) as tc:
        _emit(tc, xT, wq3, wk1, wv1, wo3, y)
    nc.compile()
    return nc


_NC = None


def _get_nc():
    global _NC
    if _NC is None:
        _NC = _build_nc()
    return _NC


def make_in_maps(x, wq, wk, wv, wo):
    x = np.asarray(x, np.float32)
    in_maps = []
    for core in range(8):
        b, kvh, sh = core >> 2, (core >> 1) & 1, core & 1
        xTb_n = x[b].T.astype(BF)                    # [D, S]
        # per-core key permutation: own query-half columns first (softmax
        # over keys is permutation invariant; outputs index queries only)
        xTb = np.concatenate(
            [xTb_n[:, sh * QC:(sh + 1) * QC],
             xTb_n[:, (1 - sh) * QC:(2 - sh) * QC]], axis=1)
        g0, g1 = kvh * GH * HD, (kvh + 1) * GH * HD

        def tile_dm(a):                              # [D, M] -> [P, NB, M]
            return np.ascontiguousarray(
                a.reshape(NB, P, a.shape[1]).transpose(1, 0, 2))

        def tile_x(a):                               # [D, M] -> [M/SC, P, NB, SC]
            return np.ascontiguousarray(
                a.reshape(NB, P, a.shape[1] // SC, SC).transpose(2, 1, 0, 3))

        in_maps.append({
            "xT": tile_x(xTb),
            "wq3": tile_dm(np.asarray(wq, np.float32)[:, g0:g1].astype(BF)),
            "wk1": tile_dm(np.asarray(wk, np.float32)[:, kvh * HD:(kvh + 1) * HD].astype(BF)),
            "wv1": tile_dm(np.asarray(wv, np.float32)[:, kvh * HD:(kvh + 1) * HD].astype(BF)),
            "wo3": np.ascontiguousarray(
                np.asarray(wo, np.float32)[g0:g1, :].astype(BF)
                .reshape(GH, P, D).transpose(1, 0, 2)),
        })
    return in_maps


def combine_outputs(results):
    """results: list of 8 per-core {name: array} dicts -> full [B, S, D] output."""
    y = np.zeros((B, S, D), np.float32)
    for b in range(B):
        for sh in range(2):
            c0 = b * 4 + 0 * 2 + sh
            c1 = b * 4 + 1 * 2 + sh
            y[b, sh * QC:(sh + 1) * QC, :] = (
                results[c0]["y"].astype(np.float32)
                + results[c1]["y"].astype(np.float32)
            )
    return y


def kernel(x, wq, wk, wv, wo, **run_kwargs):
    nc = _get_nc()
    in_maps = make_in_maps(x, wq, wk, wv, wo)
    res = run_bass_kernel_spmd(nc, in_maps, core_ids=list(range(8)), **run_kwargs)
    out = combine_outputs(res.results)
    if run_kwargs:
        kernel.last_result = res
    return out


if __name__ == "__main__":
    rng = np.random.default_rng(0)
    x = rng.standard_normal((B, S, D), dtype=np.float32)
    std = 1.0 / np.sqrt(D)
    wq = rng.standard_normal((D, N_HEADS * HD), dtype=np.float32) * std
    wk = rng.standard_normal((D, N_KV * HD), dtype=np.float32) * std
    wv = rng.standard_normal((D, N_KV * HD), dtype=np.float32) * std
    wo = rng.standard_normal((N_HEADS * HD, D), dtype=np.float32) * std
    y = kernel(x, wq, wk, wv, wo)
    print("kernel output", y.shape, y.dtype, float(np.abs(y).max()))
